# revision 1
# baseline (speedup 1.0000x reference)
"""Trainium2 Bass kernel for nn_CFTL_60327110640070.

out = x + ifft_c( fused(fft_c(mean_hw(x)), g@W1.T+b1, g@W2.T+b2) )  broadcast over HW

Strategy (pure data parallel, 8 cores, 2 samples each):
  pass 1: stream x tiles [128ch, FREE] from HBM (SP/HWDGE), DVE reduce-sum -> g
          sums; the last N_CACHE sample-0 tiles park in dedicated SBUF bufs
  stats : FFT/IFFT as 128x128-block matmuls against cos/-sin DFT matrices (PE),
          tiny elementwise chain on [128,4] tiles (DVE/ACT), interleaved into
          the sample-1 reduce stream so nothing stalls
  pass 2: re-stream x tiles (cached ones skip the reload), DVE in-place
          per-channel scalar add, store via GPSIMD/SWDGE; cached-tile adds and
          stores run early, during pass-1 of sample 1

Raw bass (no Tile): this walrus build only allows one embedded sync-wait per
DMA pseudo-instruction, so all waits are standalone wait_ge on the issuing
engine and DMAs carry only their completion-sem update. Per-ring-slot DMA
completion sems keep waited values at full totals (partial cumulative waits
race against the 16 per-engine micro-increments of in-flight DMAs).

All DFT/weight matrices are pre-transposed/pre-scaled on host so no on-device
transposes are needed (cos/-sin DFT matrices are symmetric).
"""

import sys
from contextlib import ExitStack

for _p in ("/opt/trn_rl_repo", "/root/.axon_site/_ro/trn_rl_repo"):
    if _p not in sys.path:
        sys.path.append(_p)

import numpy as np

import concourse.bass as bass
from concourse import mybir
from concourse.bass_utils import run_bass_kernel_spmd

# Problem geometry (hardcoded per contract)
N, C, H, W = 16, 512, 128, 128
HW = H * W
NCORES = 8
NS = N // NCORES          # samples per core = 2
P = 128                   # SBUF partitions
G = C // P                # channel groups = 4
FREE = 4096               # free-dim tile size for streaming x
NB_IN = 4                 # streaming ring buffers
N_CACHE = 7               # sample-0 tiles kept in SBUF across passes

_FP32 = mybir.dt.float32
_AF = mybir.ActivationFunctionType


def _build_program(free=FREE, hw=HW, nb_in=NB_IN, n_cache=N_CACHE) -> bass.Bass:
    nhalf = hw // free           # tiles per (sample, group)
    tps = G * nhalf              # x tiles (units) per sample
    n_x = NS * tps               # units per pass
    n_const = 5
    n_cache = min(n_cache, tps - 1)
    cached = list(range(tps - n_cache, tps))    # unit ids (sample 0 tail)
    is_cached = set(cached)

    # pass-2 processing order: cached units first (their adds/stores can run
    # during pass-1 of sample 1), then everything else in unit order
    p2_order = cached + [u for u in range(n_x) if u not in is_cached]
    # ring load sequence: pass-1 non-cached units, then pass-2 reloads
    ring_seq = [u for u in range(n_x) if u not in is_cached] + [
        u for u in p2_order if u not in is_cached
    ]
    ring_of_unit_p2 = {}  # unit -> ring index of its pass-2 load
    for l, u in enumerate(ring_seq):
        if l >= n_x - n_cache:
            ring_of_unit_p2[u] = l
    # sem_cons ordinals are recorded at DVE emission time (the DVE stream is
    # emitted first, so SP/GP emitters can look them up)
    cons_ct = {"n": 0}
    red_ord = {}
    add_ord = {}

    # default 16KB SWDGE descriptor-ring carveout is oversized for our ~48
    # queued stores; 8KB frees one more cache buffer's worth of SBUF
    nc = bass.Bass(dynamic_dma_scratch_size=8192)

    x_in = nc.dram_tensor("x", [NS, C, hw], _FP32, kind="ExternalInput")
    x_out = nc.dram_tensor("out", [NS, C, hw], _FP32, kind="ExternalOutput")
    # host pre-layouts: [p, g, k] with row index c = g*128+p
    cos_d = nc.dram_tensor("cosm", [P, G, C], _FP32, kind="ExternalInput")
    sin_d = nc.dram_tensor("sinn", [P, G, C], _FP32, kind="ExternalInput")
    w1_d = nc.dram_tensor("w1t", [P, G, C], _FP32, kind="ExternalInput")
    w2_d = nc.dram_tensor("w2t", [P, G, C], _FP32, kind="ExternalInput")
    b_d = nc.dram_tensor("bvec", [P, 2, G], _FP32, kind="ExternalInput")

    def unit_ap(dram, u):
        s, r = divmod(u, tps)
        cg, h = divmod(r, nhalf)
        return dram[s, cg * P:(cg + 1) * P, h * free:(h + 1) * free]

    with ExitStack() as ctx:
        sb = lambda shape, name: ctx.enter_context(
            nc.sbuf_tensor(name, shape, _FP32)
        )
        ps = lambda shape, name: ctx.enter_context(
            nc.psum_tensor(name, shape, _FP32)
        )
        sem = lambda name: ctx.enter_context(nc.semaphore(name))

        cos_sb = sb([P, G, C], "cos_sb")
        sin_sb = sb([P, G, C], "sin_sb")
        w1_sb = sb([P, G, C], "w1_sb")
        w2_sb = sb([P, G, C], "w2_sb")
        b_sb = sb([P, 2, G], "b_sb")
        halfpi = sb([P, 1], "halfpi")

        xt = [sb([P, free], f"xt{i}") for i in range(nb_in)]
        xc = [sb([P, free], f"xc{k}") for k in range(n_cache)]
        buf_of_unit = {}  # unit -> SBUF tile holding it during its add
        for k, u in enumerate(cached):
            buf_of_unit[u] = xc[k]
        for u in range(n_x):
            if u not in is_cached:
                buf_of_unit[u] = xt[ring_of_unit_p2[u] % nb_in]

        gsum = [sb([P, G, nhalf], f"gsum{s}") for s in range(NS)]
        gcol = [sb([P, G], f"gcol{s}") for s in range(NS)]
        fr = [sb([P, G], f"fr{s}") for s in range(NS)]
        fi = [sb([P, G], f"fi{s}") for s in range(NS)]
        z12 = [sb([P, 2, G], f"z12_{s}") for s in range(NS)]
        r2 = [sb([P, 2, G], f"r2_{s}") for s in range(NS)]
        s12 = [sb([P, 2, G], f"s12_{s}") for s in range(NS)]
        u0 = [sb([P, G], f"u0_{s}") for s in range(NS)]
        amp = [sb([P, G], f"amp{s}") for s in range(NS)]
        apr = [sb([P, G], f"apr{s}") for s in range(NS)]
        cosp = [sb([P, G], f"cosp{s}") for s in range(NS)]
        sinp = [sb([P, G], f"sinp{s}") for s in range(NS)]
        xi = [sb([P, G], f"xi{s}") for s in range(NS)]
        # aliases: each write is sem-ordered after the previous tenant's
        # last read (validated by the CoreSim race detector)
        u1 = amp    # u1 read by u0-add; amp written after (waits u0 done)
        ppr = fr    # fr dead after u0-mul; ppr written next
        zr = u0     # u0 dead after amp=sqrt(u0); zr written after sinp
        zi = fi     # fi dead after ppr-mul; zi written after zr

        fwd_ps = [ps([P, 4, G], f"fwd_ps{s}") for s in range(NS)]
        xi_ps = [ps([P, G], f"xi_ps{s}") for s in range(NS)]

        ld_slot = [sem(f"ld_slot{i}") for i in range(nb_in)]
        st_slot = [sem(f"st_slot{i}") for i in range(nb_in)]
        c_sem = [sem(f"c_sem{k}") for k in range(n_cache)]
        cs_sem = [sem(f"cs_sem{k}") for k in range(n_cache)]  # cached stores
        sem_cst = sem("sem_cst")    # const load completions (+16 each)
        sem_cons = sem("sem_cons")  # DVE consumed a tile (+1)
        sem_dve = sem("sem_dve")    # DVE stats milestones
        sem_act = sem("sem_act")    # ACT stats milestones
        sem_pe = sem("sem_pe")      # PE matmul groups

        # store bookkeeping: stores of ring units inc st_slot[slot]; value
        # after the c-th store on a slot is 16*c. SP's pass-2 reload of ring
        # index l waits for the consumer of ring_seq[l - nb_in]:
        #   pass-1 predecessor -> its reduce (sem_cons >= u+1)
        #   pass-2 predecessor -> its store (st_slot[slot] >= 16*count)
        store_count = [0] * nb_in
        store_val = {}  # unit (ring pass-2) -> st_slot value after its store
        for l, u in enumerate(ring_seq):
            if l >= n_x - n_cache:
                i = l % nb_in
                store_count[i] += 1
                store_val[u] = 16 * store_count[i]

        # planned sem values after named ops (any stream may reference any)
        plan = {"memset": 1}
        for s in range(NS):
            base = 2 + 10 * s  # dve count at gcol{s}
            plan[f"gcol{s}"] = base
            plan[f"z12_{s}"] = base + 1
            plan[f"s12_{s}"] = base + 2
            plan[f"u1m_{s}"] = base + 4
            plan[f"u0_{s}"] = base + 5
            plan[f"apr_{s}"] = base + 6
            plan[f"ppr_{s}"] = base + 7
            plan[f"zi_{s}"] = base + 9
            plan[f"fi_ev_{s}"] = 7 * s + 2
            plan[f"r2_{s}"] = 7 * s + 3
            plan[f"amp_{s}"] = 7 * s + 4
            plan[f"sinp_{s}"] = 7 * s + 6
            plan[f"xi_ev_{s}"] = 7 * s + 7
            plan[f"fwd_{s}"] = 2 * s + 1
            plan[f"inv_{s}"] = 2 * s + 2

        dve_v = {"n": 0}
        act_v = {"n": 0}

        with nc.Block() as block:

            @block.vector
            def _(dve):
                nv = dve_v

                def bump(tag=None):
                    nv["n"] += 1
                    if tag:
                        assert plan[tag] == nv["n"], (tag, plan[tag], nv["n"])

                nc.vector.memset(halfpi[:], float(np.pi / 2)).then_inc(sem_dve, 1)
                bump("memset")

                def reduce_unit(u):
                    s, r = divmod(u, tps)
                    cg, h = divmod(r, nhalf)
                    if u in is_cached:
                        k = cached.index(u)
                        dve.wait_ge(c_sem[k], 16)
                        src = xc[k]
                    else:
                        l = ring_seq.index(u)  # pass-1 ring index
                        dve.wait_ge(ld_slot[l % nb_in], 16 * (l // nb_in + 1))
                        src = xt[l % nb_in]
                    nc.vector.reduce_sum(
                        gsum[s][:, cg, h:h + 1], src[:],
                        axis=mybir.AxisListType.X,
                    ).then_inc(sem_cons, 1)
                    cons_ct["n"] += 1
                    red_ord[u] = cons_ct["n"]

                def gcol_reduce(s):
                    last = max(red_ord[v] for v in range(s * tps, (s + 1) * tps))
                    dve.wait_ge(sem_cons, last)  # own gsum writes done
                    nc.vector.reduce_sum(
                        gcol[s][:], gsum[s][:], axis=mybir.AxisListType.X
                    ).then_inc(sem_dve, 1)
                    bump(f"gcol{s}")

                def stats_dve(s):
                    dve.wait_ge(sem_pe, plan[f"fwd_{s}"])
                    if s == 0:
                        dve.wait_ge(sem_cst, 16 * n_const)  # b_sb resident
                    nc.vector.tensor_add(
                        z12[s][:], fwd_ps[s][:, 2:4, :], b_sb[:]
                    ).then_inc(sem_dve, 1)
                    bump(f"z12_{s}")
                    # leaky_relu(z) = z + 0.99*relu(-z)
                    dve.wait_ge(sem_act, plan[f"r2_{s}"])
                    dve.wait_ge(sem_dve, plan[f"z12_{s}"])  # self RAW
                    nc.vector.scalar_tensor_tensor(
                        out=s12[s][:], in0=r2[s][:], scalar=0.99, in1=z12[s][:],
                        op0=mybir.AluOpType.mult, op1=mybir.AluOpType.add,
                    ).then_inc(sem_dve, 1)
                    bump(f"s12_{s}")
                    dve.wait_ge(sem_act, plan[f"fi_ev_{s}"])
                    nc.vector.tensor_mul(u0[s][:], fr[s][:], fr[s][:]).then_inc(
                        sem_dve, 1
                    )
                    bump()
                    nc.vector.tensor_mul(u1[s][:], fi[s][:], fi[s][:]).then_inc(
                        sem_dve, 1
                    )
                    bump(f"u1m_{s}")
                    dve.wait_ge(sem_dve, plan[f"u1m_{s}"])  # self RAW u0/u1
                    nc.vector.tensor_add(u0[s][:], u0[s][:], u1[s][:]).then_inc(
                        sem_dve, 1
                    )
                    bump(f"u0_{s}")
                    dve.wait_ge(sem_act, plan[f"amp_{s}"])
                    dve.wait_ge(sem_dve, plan[f"s12_{s}"])  # self RAW
                    nc.vector.tensor_mul(
                        apr[s][:], s12[s][:, 0, :], amp[s][:]
                    ).then_inc(sem_dve, 1)
                    bump(f"apr_{s}")
                    nc.vector.tensor_mul(
                        ppr[s][:], s12[s][:, 1, :], fi[s][:]
                    ).then_inc(sem_dve, 1)
                    bump(f"ppr_{s}")
                    dve.wait_ge(sem_act, plan[f"sinp_{s}"])
                    dve.wait_ge(sem_dve, plan[f"apr_{s}"])  # self RAW
                    nc.vector.tensor_mul(
                        zr[s][:], apr[s][:], cosp[s][:]
                    ).then_inc(sem_dve, 1)
                    bump()
                    nc.vector.tensor_mul(
                        zi[s][:], apr[s][:], sinp[s][:]
                    ).then_inc(sem_dve, 1)
                    bump(f"zi_{s}")

                def add_unit(u, first_of_sample):
                    s, r = divmod(u, tps)
                    cg = r // nhalf
                    if first_of_sample:
                        dve.wait_ge(sem_act, plan[f"xi_ev_{s}"])
                    if u in is_cached:
                        k = cached.index(u)
                        dve.wait_ge(c_sem[k], 16)
                        buf = xc[k]
                    else:
                        l = ring_of_unit_p2[u]
                        dve.wait_ge(ld_slot[l % nb_in], 16 * (l // nb_in + 1))
                        buf = xt[l % nb_in]
                    nc.vector.tensor_scalar_add(
                        buf[:], buf[:], xi[s][:, cg:cg + 1]
                    ).then_inc(sem_cons, 1)
                    cons_ct["n"] += 1
                    add_ord[u] = cons_ct["n"]

                # ---- emission ----
                seen = [False, False]

                def add_u(u):
                    s = u // tps
                    add_unit(u, not seen[s])
                    seen[s] = True

                for u in range(tps):
                    reduce_unit(u)
                gcol_reduce(0)
                # s1 reduces with: s0 stats after ilv, cached s0 adds
                # interleaved pairwise after ilv+4 (late enough that PE/ACT
                # results are ready even on a bandwidth-starved core)
                ilv = min(8, tps - 1)
                pending = list(cached)
                for r in range(tps):
                    reduce_unit(tps + r)
                    if r + 1 == ilv:
                        stats_dve(0)
                    if r + 1 >= ilv + 4 and pending:
                        add_u(pending.pop(0))
                gcol_reduce(1)
                while pending:
                    add_u(pending.pop(0))
                rest = [u for u in p2_order if u not in is_cached]
                ilv2 = min(4, len(rest))
                for u in rest[:ilv2]:
                    add_u(u)
                stats_dve(1)
                for u in rest[ilv2:]:
                    add_u(u)

            @block.scalar
            def _(act):
                nv = act_v

                def bump(tag=None):
                    nv["n"] += 1
                    if tag:
                        assert plan[tag] == nv["n"], (tag, plan[tag], nv["n"])

                # const loads on the otherwise-idle ACT HWDGE ring so x
                # streaming starts immediately on the SP ring
                for dram, sbuf in (
                    (cos_d, cos_sb), (sin_d, sin_sb), (w1_d, w1_sb),
                    (w2_d, w2_sb), (b_d, b_sb),
                ):
                    nc.scalar.dma_start(out=sbuf[:], in_=dram[:]).then_inc(
                        sem_cst, 16
                    )
                act.wait_ge(sem_dve, plan["memset"])
                for s in range(NS):
                    act.wait_ge(sem_pe, plan[f"fwd_{s}"])
                    nc.scalar.mul(fr[s][:], fwd_ps[s][:, 0, :], 1.0 / hw)
                    bump()
                    nc.scalar.mul(fi[s][:], fwd_ps[s][:, 1, :], 1.0 / hw).then_inc(
                        sem_act, 2
                    )
                    bump(f"fi_ev_{s}")
                    act.wait_ge(sem_dve, plan[f"z12_{s}"])
                    nc.scalar.activation(
                        r2[s][:], z12[s][:], _AF.Relu, scale=-1.0
                    ).then_inc(sem_act, 1)
                    bump(f"r2_{s}")
                    act.wait_ge(sem_dve, plan[f"u0_{s}"])
                    nc.scalar.activation(amp[s][:], u0[s][:], _AF.Sqrt).then_inc(
                        sem_act, 1
                    )
                    bump(f"amp_{s}")
                    act.wait_ge(sem_dve, plan[f"ppr_{s}"])
                    nc.scalar.activation(
                        cosp[s][:], ppr[s][:], _AF.Sin, bias=halfpi[:]
                    )
                    bump()
                    nc.scalar.activation(sinp[s][:], ppr[s][:], _AF.Sin).then_inc(
                        sem_act, 2
                    )
                    bump(f"sinp_{s}")
                    act.wait_ge(sem_pe, plan[f"inv_{s}"])  # inverse mm done
                    nc.scalar.mul(xi[s][:], xi_ps[s][:], 1.0 / C).then_inc(
                        sem_act, 1
                    )
                    bump(f"xi_ev_{s}")

            @block.tensor
            def _(pe):
                pe.wait_ge(sem_cst, 16 * n_const)  # consts resident
                for s in range(NS):
                    # fwd s then inv s so xi_s lands as early as possible
                    pe.wait_ge(sem_dve, plan[f"gcol{s}"])
                    last = None
                    for t, mat in enumerate((cos_sb, sin_sb, w1_sb, w2_sb)):
                        for kg in range(G):
                            for cg in range(G):
                                last = nc.tensor.matmul(
                                    fwd_ps[s][:, t, kg:kg + 1],
                                    mat[:, cg, kg * P:(kg + 1) * P],
                                    gcol[s][:, cg:cg + 1],
                                    start=(cg == 0),
                                    stop=(cg == G - 1),
                                )
                    last.then_inc(sem_pe, 1)  # fwd_s = 2s+1
                    pe.wait_ge(sem_dve, plan[f"zi_{s}"])
                    last = None
                    for cg in range(G):
                        for kg in range(G):
                            nc.tensor.matmul(
                                xi_ps[s][:, cg:cg + 1],
                                cos_sb[:, kg, cg * P:(cg + 1) * P],
                                zr[s][:, kg:kg + 1],
                                start=(kg == 0),
                                stop=False,
                            )
                            last = nc.tensor.matmul(
                                xi_ps[s][:, cg:cg + 1],
                                sin_sb[:, kg, cg * P:(cg + 1) * P],
                                zi[s][:, kg:kg + 1],
                                start=False,
                                stop=(kg == G - 1),
                            )
                    last.then_inc(sem_pe, 1)  # inv_s = 2s+2

            @block.sync
            def _(sp):
                li = 0  # ring index
                for u in range(n_x):  # pass 1, unit order
                    if u in is_cached:
                        k = cached.index(u)
                        sp.dma_start(
                            out=xc[k][:], in_=unit_ap(x_in, u)
                        ).then_inc(c_sem[k], 16)
                        continue
                    if li >= nb_in:
                        pred = ring_seq[li - nb_in]
                        sp.wait_ge(sem_cons, red_ord[pred])  # its reduce
                    sp.dma_start(
                        out=xt[li % nb_in][:], in_=unit_ap(x_in, u)
                    ).then_inc(ld_slot[li % nb_in], 16)
                    li += 1
                for u in ring_seq[n_x - n_cache:]:  # pass 2 reloads
                    pred = ring_seq[li - nb_in]
                    if li - nb_in < n_x - n_cache:
                        sp.wait_ge(sem_cons, red_ord[pred])  # pred's reduce
                    else:
                        sp.wait_ge(st_slot[li % nb_in], store_val[pred])
                    sp.dma_start(
                        out=xt[li % nb_in][:], in_=unit_ap(x_in, u)
                    ).then_inc(ld_slot[li % nb_in], 16)
                    li += 1

            @block.gpsimd
            def _(gp):
                for q, u in enumerate(p2_order):
                    gp.wait_ge(sem_cons, add_ord[u])
                    d = gp.dma_start(
                        out=unit_ap(x_out, u), in_=buf_of_unit[u][:]
                    )
                    if u in is_cached:
                        d.then_inc(cs_sem[cached.index(u)], 16)  # unwaited
                    else:
                        i = ring_of_unit_p2[u] % nb_in
                        d.then_inc(st_slot[i], 16)

    return nc


_NC_CACHE = None


def _get_program():
    global _NC_CACHE
    if _NC_CACHE is None:
        _NC_CACHE = _build_program()
    return _NC_CACHE


def _host_constants():
    idx = np.arange(C)
    th = (2.0 * np.pi / C) * np.outer(idx, idx)
    cosm = np.cos(th).astype(np.float32)
    sinn = (-np.sin(th)).astype(np.float32)
    # [p, g, k] layout with row c = g*128+p
    to_pgk = lambda m: np.ascontiguousarray(m.reshape(G, P, C).transpose(1, 0, 2))
    return to_pgk(cosm), to_pgk(sinn)


_CONSTS_CACHE = None


def make_in_maps(inputs, hw=HW):
    """Shard + preprocess inputs into 8 per-core input maps."""
    global _CONSTS_CACHE
    if _CONSTS_CACHE is None:
        _CONSTS_CACHE = _host_constants()
    cos_pgk, sin_pgk = _CONSTS_CACHE

    x = np.ascontiguousarray(inputs["x"], dtype=np.float32)
    W1 = np.asarray(inputs["W1"], dtype=np.float32)
    W2 = np.asarray(inputs["W2"], dtype=np.float32)
    b1 = np.asarray(inputs["b1"], dtype=np.float32)
    b2 = np.asarray(inputs["b2"], dtype=np.float32)

    # fold the 1/HW mean normalization into the linear-layer weights
    w1t = np.ascontiguousarray(
        (W1.T / hw).reshape(G, P, C).transpose(1, 0, 2), dtype=np.float32
    )
    w2t = np.ascontiguousarray(
        (W2.T / hw).reshape(G, P, C).transpose(1, 0, 2), dtype=np.float32
    )
    bvec = np.ascontiguousarray(
        np.stack([b1.reshape(G, P), b2.reshape(G, P)]).transpose(2, 0, 1),
        dtype=np.float32,
    )  # [P, 2, G]

    xs = x.reshape(NCORES, NS, C, hw)
    return [
        {
            "x": xs[i],
            "cosm": cos_pgk,
            "sinn": sin_pgk,
            "w1t": w1t,
            "w2t": w2t,
            "bvec": bvec,
        }
        for i in range(NCORES)
    ]


def _run(inputs, trace=False, trace_kwargs=None):
    in_maps = make_in_maps(inputs)
    nc = _get_program()
    res = run_bass_kernel_spmd(
        nc,
        in_maps,
        list(range(NCORES)),
        trace=trace,
        **(trace_kwargs or {}),
    )
    out = np.stack([r["out"] for r in res.results])
    return out.reshape(N, C, H, W).astype(np.float32), res


def kernel(**inputs) -> np.ndarray:
    out, _ = _run(inputs, trace=False)
    return out



# revision 2
# speedup vs baseline: 1.8030x; 1.8030x over previous
"""Trainium2 Bass kernel for nn_CFTL_60327110640070.

out = x + ifft_c( fused(fft_c(mean_hw(x)), g@W1.T+b1, g@W2.T+b2) )  broadcast over HW

Strategy (pure data parallel, 8 cores, 2 samples each, fp16 streaming):
  x is uploaded to the device as fp16 (halves load traffic; rel-err ~2e-4
  is far inside the 2e-2 gate) and the output is written as fp16 and
  upcast to fp32 on the host (halves store traffic). A full sample
  (16 x [128,4096] fp16 tiles = 128 KiB/partition) stays resident in
  SBUF, so x is read exactly once -- no second pass:

  per sample: load 16 tiles -> DVE reduce each -> gcol -> PE DFT/linear
  matmuls -> DVE/ACT stats chain -> xi -> DVE in-place add -> GP store.
  Sample 1's first 5 tiles load into spare buffers during sample 0's
  stats/adds; its remaining 11 reuse sample-0 buffers as stores drain.

Raw bass (no Tile): all waits are standalone wait_ge on the issuing
engine; DMAs carry only their completion-sem update. Each DMA sem has at
most one in-flight DMA at a time (enforced by the data deps), so
cumulative 16*k waits are race-free against the 16 per-engine
micro-increments.

All DFT/weight matrices are pre-transposed/pre-scaled on host so no
on-device transposes are needed (cos/-sin DFT matrices are symmetric).
"""

import sys
from contextlib import ExitStack

for _p in ("/opt/trn_rl_repo", "/root/.axon_site/_ro/trn_rl_repo"):
    if _p not in sys.path:
        sys.path.append(_p)

import numpy as np

import concourse.bass as bass
from concourse import mybir
from concourse.bass_utils import run_bass_kernel_spmd

# Problem geometry (hardcoded per contract)
N, C, H, W = 16, 512, 128, 128
HW = H * W
NCORES = 8
NS = N // NCORES          # samples per core = 2
P = 128                   # SBUF partitions
G = C // P                # channel groups = 4
FREE = 4096               # free-dim tile size for streaming x
NSPARE = 5                # extra unit buffers for cross-sample overlap

_FP32 = mybir.dt.float32
_FP16 = mybir.dt.float16
_AF = mybir.ActivationFunctionType


def _build_program(free=FREE, hw=HW, nspare=NSPARE) -> bass.Bass:
    nhalf = hw // free           # tiles per (sample, group) = 4
    tps = G * nhalf              # x tiles (units) per sample = 16
    n_const = 5
    nbuf = tps + nspare          # 21 unit buffers

    # buffer assignment: s0 units -> bufs 0..tps-1; s1 units 0..nspare-1 ->
    # spare bufs; s1 units nspare.. -> bufs 0..tps-nspare-1 (after s0 store)
    buf_of = {}
    for u in range(tps):
        buf_of[(0, u)] = u
    for u in range(nspare):
        buf_of[(1, u)] = tps + u
    for u in range(nspare, tps):
        buf_of[(1, u)] = u - nspare

    cons_ct = {"n": 0}
    red_ord = {}
    add_ord = {}

    nc = bass.Bass(dynamic_dma_scratch_size=8192)

    x_in = nc.dram_tensor("x", [NS, C, hw], _FP16, kind="ExternalInput")
    x_out = nc.dram_tensor("out", [NS, C, hw], _FP16, kind="ExternalOutput")
    # host pre-layouts: [p, g, k] with row index c = g*128+p
    cos_d = nc.dram_tensor("cosm", [P, G, C], _FP32, kind="ExternalInput")
    sin_d = nc.dram_tensor("sinn", [P, G, C], _FP32, kind="ExternalInput")
    w1_d = nc.dram_tensor("w1t", [P, G, C], _FP32, kind="ExternalInput")
    w2_d = nc.dram_tensor("w2t", [P, G, C], _FP32, kind="ExternalInput")
    b_d = nc.dram_tensor("bvec", [P, 2, G], _FP32, kind="ExternalInput")

    def unit_ap(dram, s, u):
        cg, h = divmod(u, nhalf)
        return dram[s, cg * P:(cg + 1) * P, h * free:(h + 1) * free]

    with ExitStack() as ctx:
        sb = lambda shape, name, dt=_FP32: ctx.enter_context(
            nc.sbuf_tensor(name, shape, dt)
        )
        ps = lambda shape, name: ctx.enter_context(
            nc.psum_tensor(name, shape, _FP32)
        )
        sem = lambda name: ctx.enter_context(nc.semaphore(name))

        cos_sb = sb([P, G, C], "cos_sb")
        sin_sb = sb([P, G, C], "sin_sb")
        w1_sb = sb([P, G, C], "w1_sb")
        w2_sb = sb([P, G, C], "w2_sb")
        b_sb = sb([P, 2, G], "b_sb")
        halfpi = sb([P, 1], "halfpi")

        xb = [sb([P, free], f"xb{i}", _FP16) for i in range(nbuf)]

        gsum = [sb([P, G, nhalf], f"gsum{s}") for s in range(NS)]
        gcol = [sb([P, G], f"gcol{s}") for s in range(NS)]
        fr = [sb([P, G], f"fr{s}") for s in range(NS)]
        fi = [sb([P, G], f"fi{s}") for s in range(NS)]
        z12 = [sb([P, 2, G], f"z12_{s}") for s in range(NS)]
        r2 = [sb([P, 2, G], f"r2_{s}") for s in range(NS)]
        s12 = [sb([P, 2, G], f"s12_{s}") for s in range(NS)]
        u0 = [sb([P, G], f"u0_{s}") for s in range(NS)]
        amp = [sb([P, G], f"amp{s}") for s in range(NS)]
        apr = [sb([P, G], f"apr{s}") for s in range(NS)]
        cosp = [sb([P, G], f"cosp{s}") for s in range(NS)]
        sinp = [sb([P, G], f"sinp{s}") for s in range(NS)]
        xi = [sb([P, G], f"xi{s}") for s in range(NS)]
        # aliases: each write is sem-ordered after the previous tenant's
        # last read (same per-sample op order as validated baseline)
        u1 = amp    # u1 read by u0-add; amp written after (waits u0 done)
        ppr = fr    # fr dead after u0-mul; ppr written next
        zr = u0     # u0 dead after amp=sqrt(u0); zr written after sinp
        zi = fi     # fi dead after ppr-mul; zi written after zr

        fwd_ps = [ps([P, 4, G], f"fwd_ps{s}") for s in range(NS)]
        xi_ps = [ps([P, G], f"xi_ps{s}") for s in range(NS)]

        ld = [sem(f"ld{b}") for b in range(nbuf)]
        st = [sem(f"st{b}") for b in range(tps - nspare)]  # waited reloads
        st_misc = sem("st_misc")    # unwaited store completions
        sem_cst = sem("sem_cst")    # const load completions (+16 each)
        sem_cons = sem("sem_cons")  # DVE consumed a tile (+1)
        sem_dve = sem("sem_dve")    # DVE stats milestones
        sem_act = sem("sem_act")    # ACT stats milestones
        sem_pe = sem("sem_pe")      # PE matmul groups

        # planned sem values after named ops (any stream may reference any)
        plan = {"memset": 1}
        for s in range(NS):
            base = 2 + 10 * s  # dve count at gcol{s}
            plan[f"gcol{s}"] = base
            plan[f"z12_{s}"] = base + 1
            plan[f"s12_{s}"] = base + 2
            plan[f"u1m_{s}"] = base + 4
            plan[f"u0_{s}"] = base + 5
            plan[f"apr_{s}"] = base + 6
            plan[f"ppr_{s}"] = base + 7
            plan[f"zi_{s}"] = base + 9
            plan[f"fi_ev_{s}"] = 7 * s + 2
            plan[f"r2_{s}"] = 7 * s + 3
            plan[f"amp_{s}"] = 7 * s + 4
            plan[f"sinp_{s}"] = 7 * s + 6
            plan[f"xi_ev_{s}"] = 7 * s + 7
            plan[f"fwd_{s}"] = 2 * s + 1
            plan[f"inv_{s}"] = 2 * s + 2

        dve_v = {"n": 0}
        act_v = {"n": 0}

        with nc.Block() as block:

            @block.vector
            def _(dve):
                nv = dve_v

                def bump(tag=None):
                    nv["n"] += 1
                    if tag:
                        assert plan[tag] == nv["n"], (tag, plan[tag], nv["n"])

                nc.vector.memset(halfpi[:], float(np.pi / 2)).then_inc(sem_dve, 1)
                bump("memset")

                def reduce_unit(s, u):
                    b = buf_of[(s, u)]
                    cg, h = divmod(u, nhalf)
                    # second-generation loads (s1 reusing an s0 buf) -> 32
                    gen2 = s == 1 and u >= nspare
                    dve.wait_ge(ld[b], 32 if gen2 else 16)
                    nc.vector.reduce_sum(
                        gsum[s][:, cg, h:h + 1], xb[b][:],
                        axis=mybir.AxisListType.X,
                    ).then_inc(sem_cons, 1)
                    cons_ct["n"] += 1
                    red_ord[(s, u)] = cons_ct["n"]

                def gcol_reduce(s):
                    last = max(red_ord[(s, v)] for v in range(tps))
                    dve.wait_ge(sem_cons, last)  # own gsum writes done
                    nc.vector.reduce_sum(
                        gcol[s][:], gsum[s][:], axis=mybir.AxisListType.X
                    ).then_inc(sem_dve, 1)
                    bump(f"gcol{s}")

                def stats_dve(s):
                    dve.wait_ge(sem_pe, plan[f"fwd_{s}"])
                    if s == 0:
                        dve.wait_ge(sem_cst, 16 * n_const)  # b_sb resident
                    nc.vector.tensor_add(
                        z12[s][:], fwd_ps[s][:, 2:4, :], b_sb[:]
                    ).then_inc(sem_dve, 1)
                    bump(f"z12_{s}")
                    # leaky_relu(z) = z + 0.99*relu(-z)
                    dve.wait_ge(sem_act, plan[f"r2_{s}"])
                    dve.wait_ge(sem_dve, plan[f"z12_{s}"])  # self RAW
                    nc.vector.scalar_tensor_tensor(
                        out=s12[s][:], in0=r2[s][:], scalar=0.99, in1=z12[s][:],
                        op0=mybir.AluOpType.mult, op1=mybir.AluOpType.add,
                    ).then_inc(sem_dve, 1)
                    bump(f"s12_{s}")
                    dve.wait_ge(sem_act, plan[f"fi_ev_{s}"])
                    nc.vector.tensor_mul(u0[s][:], fr[s][:], fr[s][:]).then_inc(
                        sem_dve, 1
                    )
                    bump()
                    nc.vector.tensor_mul(u1[s][:], fi[s][:], fi[s][:]).then_inc(
                        sem_dve, 1
                    )
                    bump(f"u1m_{s}")
                    dve.wait_ge(sem_dve, plan[f"u1m_{s}"])  # self RAW u0/u1
                    nc.vector.tensor_add(u0[s][:], u0[s][:], u1[s][:]).then_inc(
                        sem_dve, 1
                    )
                    bump(f"u0_{s}")
                    dve.wait_ge(sem_act, plan[f"amp_{s}"])
                    dve.wait_ge(sem_dve, plan[f"s12_{s}"])  # self RAW
                    nc.vector.tensor_mul(
                        apr[s][:], s12[s][:, 0, :], amp[s][:]
                    ).then_inc(sem_dve, 1)
                    bump(f"apr_{s}")
                    nc.vector.tensor_mul(
                        ppr[s][:], s12[s][:, 1, :], fi[s][:]
                    ).then_inc(sem_dve, 1)
                    bump(f"ppr_{s}")
                    dve.wait_ge(sem_act, plan[f"sinp_{s}"])
                    dve.wait_ge(sem_dve, plan[f"apr_{s}"])  # self RAW
                    nc.vector.tensor_mul(
                        zr[s][:], apr[s][:], cosp[s][:]
                    ).then_inc(sem_dve, 1)
                    bump()
                    nc.vector.tensor_mul(
                        zi[s][:], apr[s][:], sinp[s][:]
                    ).then_inc(sem_dve, 1)
                    bump(f"zi_{s}")

                def add_unit(s, u, first_of_sample):
                    b = buf_of[(s, u)]
                    cg = u // nhalf
                    if first_of_sample:
                        dve.wait_ge(sem_act, plan[f"xi_ev_{s}"])
                    nc.vector.tensor_scalar_add(
                        xb[b][:], xb[b][:], xi[s][:, cg:cg + 1]
                    ).then_inc(sem_cons, 1)
                    cons_ct["n"] += 1
                    add_ord[(s, u)] = cons_ct["n"]

                # ---- emission ----
                for u in range(tps):
                    reduce_unit(0, u)
                gcol_reduce(0)
                # s1 spare-buffer reduces while PE/ACT run sample-0 stats
                for u in range(nspare):
                    reduce_unit(1, u)
                stats_dve(0)
                for u in range(tps):
                    add_unit(0, u, u == 0)
                for u in range(nspare, tps):
                    reduce_unit(1, u)
                gcol_reduce(1)
                stats_dve(1)
                for u in range(tps):
                    add_unit(1, u, u == 0)

            @block.scalar
            def _(act):
                nv = act_v

                def bump(tag=None):
                    nv["n"] += 1
                    if tag:
                        assert plan[tag] == nv["n"], (tag, plan[tag], nv["n"])

                # const loads on the otherwise-idle ACT HWDGE ring so x
                # streaming starts immediately on the SP ring
                for dram, sbuf in (
                    (cos_d, cos_sb), (sin_d, sin_sb), (w1_d, w1_sb),
                    (w2_d, w2_sb), (b_d, b_sb),
                ):
                    nc.scalar.dma_start(out=sbuf[:], in_=dram[:]).then_inc(
                        sem_cst, 16
                    )
                act.wait_ge(sem_dve, plan["memset"])
                for s in range(NS):
                    act.wait_ge(sem_pe, plan[f"fwd_{s}"])
                    nc.scalar.mul(fr[s][:], fwd_ps[s][:, 0, :], 1.0 / hw)
                    bump()
                    nc.scalar.mul(fi[s][:], fwd_ps[s][:, 1, :], 1.0 / hw).then_inc(
                        sem_act, 2
                    )
                    bump(f"fi_ev_{s}")
                    act.wait_ge(sem_dve, plan[f"z12_{s}"])
                    nc.scalar.activation(
                        r2[s][:], z12[s][:], _AF.Relu, scale=-1.0
                    ).then_inc(sem_act, 1)
                    bump(f"r2_{s}")
                    act.wait_ge(sem_dve, plan[f"u0_{s}"])
                    nc.scalar.activation(amp[s][:], u0[s][:], _AF.Sqrt).then_inc(
                        sem_act, 1
                    )
                    bump(f"amp_{s}")
                    act.wait_ge(sem_dve, plan[f"ppr_{s}"])
                    nc.scalar.activation(
                        cosp[s][:], ppr[s][:], _AF.Sin, bias=halfpi[:]
                    )
                    bump()
                    nc.scalar.activation(sinp[s][:], ppr[s][:], _AF.Sin).then_inc(
                        sem_act, 2
                    )
                    bump(f"sinp_{s}")
                    act.wait_ge(sem_pe, plan[f"inv_{s}"])  # inverse mm done
                    nc.scalar.mul(xi[s][:], xi_ps[s][:], 1.0 / C).then_inc(
                        sem_act, 1
                    )
                    bump(f"xi_ev_{s}")

            @block.tensor
            def _(pe):
                pe.wait_ge(sem_cst, 16 * n_const)  # consts resident
                for s in range(NS):
                    # fwd s then inv s so xi_s lands as early as possible
                    pe.wait_ge(sem_dve, plan[f"gcol{s}"])
                    last = None
                    for t, mat in enumerate((cos_sb, sin_sb, w1_sb, w2_sb)):
                        for kg in range(G):
                            for cg in range(G):
                                last = nc.tensor.matmul(
                                    fwd_ps[s][:, t, kg:kg + 1],
                                    mat[:, cg, kg * P:(kg + 1) * P],
                                    gcol[s][:, cg:cg + 1],
                                    start=(cg == 0),
                                    stop=(cg == G - 1),
                                )
                    last.then_inc(sem_pe, 1)  # fwd_s = 2s+1
                    pe.wait_ge(sem_dve, plan[f"zi_{s}"])
                    last = None
                    for cg in range(G):
                        for kg in range(G):
                            nc.tensor.matmul(
                                xi_ps[s][:, cg:cg + 1],
                                cos_sb[:, kg, cg * P:(cg + 1) * P],
                                zr[s][:, kg:kg + 1],
                                start=(kg == 0),
                                stop=False,
                            )
                            last = nc.tensor.matmul(
                                xi_ps[s][:, cg:cg + 1],
                                sin_sb[:, kg, cg * P:(cg + 1) * P],
                                zi[s][:, kg:kg + 1],
                                start=False,
                                stop=(kg == G - 1),
                            )
                    last.then_inc(sem_pe, 1)  # inv_s = 2s+2

            @block.sync
            def _(sp):
                for u in range(tps):  # sample 0
                    sp.dma_start(
                        out=xb[u][:], in_=unit_ap(x_in, 0, u)
                    ).then_inc(ld[u], 16)
                for u in range(nspare):  # sample 1 head -> spare bufs
                    sp.dma_start(
                        out=xb[tps + u][:], in_=unit_ap(x_in, 1, u)
                    ).then_inc(ld[tps + u], 16)
                for u in range(nspare, tps):  # sample 1 tail -> reused bufs
                    b = u - nspare
                    sp.wait_ge(st[b], 16)  # s0's store from buf b done
                    sp.dma_start(
                        out=xb[b][:], in_=unit_ap(x_in, 1, u)
                    ).then_inc(ld[b], 16)

            @block.gpsimd
            def _(gp):
                for s in range(NS):
                    for u in range(tps):
                        b = buf_of[(s, u)]
                        gp.wait_ge(sem_cons, add_ord[(s, u)])
                        d = gp.dma_start(
                            out=unit_ap(x_out, s, u), in_=xb[b][:]
                        )
                        if s == 0 and b < tps - nspare:
                            d.then_inc(st[b], 16)  # unblocks s1's reload
                        else:
                            d.then_inc(st_misc, 16)  # unwaited

    return nc


_NC_CACHE = None


def _get_program():
    global _NC_CACHE
    if _NC_CACHE is None:
        _NC_CACHE = _build_program()
    return _NC_CACHE


def _host_constants():
    idx = np.arange(C)
    th = (2.0 * np.pi / C) * np.outer(idx, idx)
    cosm = np.cos(th).astype(np.float32)
    sinn = (-np.sin(th)).astype(np.float32)
    # [p, g, k] layout with row c = g*128+p
    to_pgk = lambda m: np.ascontiguousarray(m.reshape(G, P, C).transpose(1, 0, 2))
    return to_pgk(cosm), to_pgk(sinn)


_CONSTS_CACHE = None


def make_in_maps(inputs, hw=HW):
    """Shard + preprocess inputs into 8 per-core input maps."""
    global _CONSTS_CACHE
    if _CONSTS_CACHE is None:
        _CONSTS_CACHE = _host_constants()
    cos_pgk, sin_pgk = _CONSTS_CACHE

    x = np.asarray(inputs["x"])
    W1 = np.asarray(inputs["W1"], dtype=np.float32)
    W2 = np.asarray(inputs["W2"], dtype=np.float32)
    b1 = np.asarray(inputs["b1"], dtype=np.float32)
    b2 = np.asarray(inputs["b2"], dtype=np.float32)

    # fold the 1/HW mean normalization into the linear-layer weights
    w1t = np.ascontiguousarray(
        (W1.T / hw).reshape(G, P, C).transpose(1, 0, 2), dtype=np.float32
    )
    w2t = np.ascontiguousarray(
        (W2.T / hw).reshape(G, P, C).transpose(1, 0, 2), dtype=np.float32
    )
    bvec = np.ascontiguousarray(
        np.stack([b1.reshape(G, P), b2.reshape(G, P)]).transpose(2, 0, 1),
        dtype=np.float32,
    )  # [P, 2, G]

    xs = np.ascontiguousarray(x, dtype=np.float16).reshape(NCORES, NS, C, hw)
    return [
        {
            "x": xs[i],
            "cosm": cos_pgk,
            "sinn": sin_pgk,
            "w1t": w1t,
            "w2t": w2t,
            "bvec": bvec,
        }
        for i in range(NCORES)
    ]


def _run(inputs, trace=False, trace_kwargs=None):
    in_maps = make_in_maps(inputs)
    nc = _get_program()
    res = run_bass_kernel_spmd(
        nc,
        in_maps,
        list(range(NCORES)),
        trace=trace,
        **(trace_kwargs or {}),
    )
    out = np.stack([r["out"] for r in res.results])
    return out.reshape(N, C, H, W).astype(np.float32), res


def kernel(**inputs) -> np.ndarray:
    out, _ = _run(inputs, trace=False)
    return out


# revision 10
# speedup vs baseline: 2.2952x; 1.2730x over previous
"""Trainium2 Bass kernel for nn_CFTL_60327110640070.

out = x + ifft_c( fused(fft_c(mean_hw(x)), g@W1.T+b1, g@W2.T+b2) )  broadcast over HW

Strategy (pure data parallel, 8 cores, 2 samples each, fp16 streaming):
  x is uploaded to the device as fp16 (halves load traffic; rel-err ~2e-4
  is far inside the 2e-2 gate) and the output is written as fp16 and
  upcast to fp32 on the host (halves store traffic). A full sample
  (16 x [128,4096] fp16 tiles = 128 KiB/partition) stays resident in
  SBUF, so x is read exactly once -- no second pass:

  per sample: load 16 tiles -> DVE reduce each -> gcol -> PE DFT/linear
  matmuls (bf16) -> DVE/ACT stats chain -> xi -> DVE in-place add -> GP
  store. Sample 1's first 7 tiles load into spare buffers during sample
  0's stats/adds; its remaining 9 reuse sample-0 buffers as stores drain.

  Per-unit HW-reduction runs as tensor_scalar(+0.0, accum_out=sum) --
  InstTensorReduce has no DVE 2x mode (3.7us/tile) while tensor_scalar
  on packed fp16 does (1.2us/tile). DFT/weight matmuls in bf16
  (fp32 PE runs as 2 half-speed passes; bf16 is 4x).

Raw bass (no Tile): all waits are standalone wait_ge on the issuing
engine; DMAs carry only their completion-sem update. Each DMA sem has at
most one in-flight DMA at a time (enforced by the data deps), so
cumulative 16*k waits are race-free against the 16 per-engine
micro-increments.

All DFT/weight matrices are pre-transposed/pre-scaled on host so no
on-device transposes are needed (cos/-sin DFT matrices are symmetric).
"""

import sys
from contextlib import ExitStack

for _p in ("/opt/trn_rl_repo", "/root/.axon_site/_ro/trn_rl_repo"):
    if _p not in sys.path:
        sys.path.append(_p)

import numpy as np

import concourse.bass as bass
from concourse import mybir
from concourse.bass_utils import run_bass_kernel_spmd

# Problem geometry (hardcoded per contract)
N, C, H, W = 16, 512, 128, 128
HW = H * W
NCORES = 8
NS = N // NCORES          # samples per core = 2
P = 128                   # SBUF partitions
G = C // P                # channel groups = 4
FREE = 4096               # free-dim tile size for streaming x
NSPARE = 7                # extra unit buffers for cross-sample overlap

_FP32 = mybir.dt.float32
_FP16 = mybir.dt.float16
_BF16 = mybir.dt.bfloat16
_AF = mybir.ActivationFunctionType
_NP_BF16 = np.dtype(mybir.dt.np(_BF16))


def _build_program(free=FREE, hw=HW, nspare=NSPARE) -> bass.Bass:
    nhalf = hw // free           # tiles per (sample, group) = 4
    tps = G * nhalf              # x tiles (units) per sample = 16
    n_const = 5
    nbuf = tps + nspare          # 23 unit buffers

    # buffer assignment: s0 units -> bufs 0..tps-1; s1 units 0..nspare-1 ->
    # spare bufs; s1 units nspare.. -> bufs 0..tps-nspare-1 (after s0 store)
    buf_of = {}
    for u in range(tps):
        buf_of[(0, u)] = u
    for u in range(nspare):
        buf_of[(1, u)] = tps + u
    for u in range(nspare, tps):
        buf_of[(1, u)] = u - nspare

    cons_ct = {"n": 0}
    red_ord = {}
    add_ord = {}

    nc = bass.Bass(dynamic_dma_scratch_size=8192)

    x_in = nc.dram_tensor("x", [NS, C, hw], _FP16, kind="ExternalInput")
    x_out = nc.dram_tensor("out", [NS, C, hw], _FP16, kind="ExternalOutput")
    # host pre-layouts: [p, g, k] with row index c = g*128+p
    cos_d = nc.dram_tensor("cosm", [P, G, C], _BF16, kind="ExternalInput")
    sin_d = nc.dram_tensor("sinn", [P, G, C], _BF16, kind="ExternalInput")
    w1_d = nc.dram_tensor("w1t", [P, G, C], _BF16, kind="ExternalInput")
    w2_d = nc.dram_tensor("w2t", [P, G, C], _BF16, kind="ExternalInput")
    b_d = nc.dram_tensor("bvec", [P, 2, G], _FP32, kind="ExternalInput")

    def unit_ap(dram, s, u):
        cg, h = divmod(u, nhalf)
        return dram[s, cg * P:(cg + 1) * P, h * free:(h + 1) * free]

    with ExitStack() as ctx:
        sb = lambda shape, name, dt=_FP32: ctx.enter_context(
            nc.sbuf_tensor(name, shape, dt)
        )
        ps = lambda shape, name: ctx.enter_context(
            nc.psum_tensor(name, shape, _FP32)
        )
        sem = lambda name: ctx.enter_context(nc.semaphore(name))

        cos_sb = sb([P, G, C], "cos_sb", _BF16)
        sin_sb = sb([P, G, C], "sin_sb", _BF16)
        w1_sb = sb([P, G, C], "w1_sb", _BF16)
        w2_sb = sb([P, G, C], "w2_sb", _BF16)
        b_sb = sb([P, 2, G], "b_sb")
        halfpi = sb([P, 1], "halfpi")

        xb = [sb([P, free], f"xb{i}", _FP16) for i in range(nbuf)]

        gsum = [sb([P, G, nhalf], f"gsum{s}") for s in range(NS)]
        gcol = [sb([P, G], f"gcol{s}", _BF16) for s in range(NS)]
        fr = [sb([P, G], f"fr{s}") for s in range(NS)]
        fi = [sb([P, G], f"fi{s}") for s in range(NS)]
        z12 = [sb([P, 2, G], f"z12_{s}") for s in range(NS)]
        r2 = [sb([P, 2, G], f"r2_{s}") for s in range(NS)]
        s12 = [sb([P, 2, G], f"s12_{s}") for s in range(NS)]
        u0 = [sb([P, G], f"u0_{s}") for s in range(NS)]
        amp = [sb([P, G], f"amp{s}") for s in range(NS)]
        apr = [sb([P, G], f"apr{s}") for s in range(NS)]
        cosp = [sb([P, G], f"cosp{s}") for s in range(NS)]
        sinp = [sb([P, G], f"sinp{s}") for s in range(NS)]
        xi = [sb([P, G], f"xi{s}") for s in range(NS)]
        zr = [sb([P, G], f"zr{s}", _BF16) for s in range(NS)]
        zi = [sb([P, G], f"zi{s}", _BF16) for s in range(NS)]
        # aliases: each write is sem-ordered after the previous tenant's
        # last read (same per-sample op order as validated baseline)
        u1 = amp    # u1 read by u0-add; amp written after (waits u0 done)
        ppr = fr    # fr dead after u0-mul; ppr written next

        fwd_ps = [ps([P, 4, G], f"fwd_ps{s}") for s in range(NS)]
        xi_ps = [ps([P, G], f"xi_ps{s}") for s in range(NS)]

        ld = [sem(f"ld{b}") for b in range(nbuf)]
        st = [sem(f"st{b}") for b in range(tps - nspare)]  # waited reloads
        st_misc = sem("st_misc")    # unwaited store completions
        sem_cst = sem("sem_cst")    # const load completions (+16 each)
        sem_cons = sem("sem_cons")  # DVE consumed a tile (+1)
        sem_dve = sem("sem_dve")    # DVE stats milestones
        sem_act = sem("sem_act")    # ACT stats milestones
        sem_pe = sem("sem_pe")      # PE matmul groups

        # planned sem values after named ops (any stream may reference any)
        plan = {"memset": 1}
        for s in range(NS):
            base = 2 + 10 * s  # dve count at gcol{s}
            plan[f"gcol{s}"] = base
            plan[f"z12_{s}"] = base + 1
            plan[f"s12_{s}"] = base + 2
            plan[f"u1m_{s}"] = base + 4
            plan[f"u0_{s}"] = base + 5
            plan[f"apr_{s}"] = base + 6
            plan[f"ppr_{s}"] = base + 7
            plan[f"zi_{s}"] = base + 9
            plan[f"fi_ev_{s}"] = 7 * s + 2
            plan[f"r2_{s}"] = 7 * s + 3
            plan[f"amp_{s}"] = 7 * s + 4
            plan[f"sinp_{s}"] = 7 * s + 6
            plan[f"xi_ev_{s}"] = 7 * s + 7
            plan[f"fwd_{s}"] = 2 * s + 1
            plan[f"inv_{s}"] = 2 * s + 2

        dve_v = {"n": 0}
        act_v = {"n": 0}

        with nc.Block() as block:

            @block.vector
            def _(dve):
                nv = dve_v

                def bump(tag=None):
                    nv["n"] += 1
                    if tag:
                        assert plan[tag] == nv["n"], (tag, plan[tag], nv["n"])

                nc.vector.memset(halfpi[:], float(np.pi / 2)).then_inc(sem_dve, 1)
                bump("memset")

                def reduce_unit(s, u):
                    b = buf_of[(s, u)]
                    cg, h = divmod(u, nhalf)
                    # second-generation loads (s1 reusing an s0 buf) -> 32
                    gen2 = s == 1 and u >= nspare
                    dve.wait_ge(ld[b], 32 if gen2 else 16)
                    # HW-sum via tensor_scalar(+0.0) accum_out: the identity
                    # write keeps the fast packed-fp16 DVE path (plain
                    # InstTensorReduce has no 2x mode and is ~3x slower);
                    # op1 is the TensorScalarPtrReduce reduce op, scalar2
                    # its init value (validated bit-preserving on HW)
                    nc.vector.tensor_scalar(
                        out=xb[b][:], in0=xb[b][:], scalar1=0.0, scalar2=0.0,
                        op0=mybir.AluOpType.add, op1=mybir.AluOpType.add,
                        accum_out=gsum[s][:, cg, h:h + 1],
                    ).then_inc(sem_cons, 1)
                    cons_ct["n"] += 1
                    red_ord[(s, u)] = cons_ct["n"]

                def gcol_reduce(s):
                    last = max(red_ord[(s, v)] for v in range(tps))
                    dve.wait_ge(sem_cons, last)  # own gsum writes done
                    # bf16 out is deliberate: gcol feeds bf16 PE matmuls;
                    # the 4-way accumulate itself still runs in fp32
                    with nc.allow_low_precision(reason="bf16 gcol for bf16 PE"):
                        nc.vector.reduce_sum(
                            gcol[s][:], gsum[s][:], axis=mybir.AxisListType.X
                        ).then_inc(sem_dve, 1)
                    bump(f"gcol{s}")

                def stats_dve(s):
                    dve.wait_ge(sem_pe, plan[f"fwd_{s}"])
                    if s == 0:
                        dve.wait_ge(sem_cst, 16 * n_const)  # b_sb resident
                    nc.vector.tensor_add(
                        z12[s][:], fwd_ps[s][:, 2:4, :], b_sb[:]
                    ).then_inc(sem_dve, 1)
                    bump(f"z12_{s}")
                    # leaky_relu(z) = z + 0.99*relu(-z)
                    dve.wait_ge(sem_act, plan[f"r2_{s}"])
                    dve.wait_ge(sem_dve, plan[f"z12_{s}"])  # self RAW
                    nc.vector.scalar_tensor_tensor(
                        out=s12[s][:], in0=r2[s][:], scalar=0.99, in1=z12[s][:],
                        op0=mybir.AluOpType.mult, op1=mybir.AluOpType.add,
                    ).then_inc(sem_dve, 1)
                    bump(f"s12_{s}")
                    dve.wait_ge(sem_act, plan[f"fi_ev_{s}"])
                    nc.vector.tensor_mul(u0[s][:], fr[s][:], fr[s][:]).then_inc(
                        sem_dve, 1
                    )
                    bump()
                    nc.vector.tensor_mul(u1[s][:], fi[s][:], fi[s][:]).then_inc(
                        sem_dve, 1
                    )
                    bump(f"u1m_{s}")
                    dve.wait_ge(sem_dve, plan[f"u1m_{s}"])  # self RAW u0/u1
                    nc.vector.tensor_add(u0[s][:], u0[s][:], u1[s][:]).then_inc(
                        sem_dve, 1
                    )
                    bump(f"u0_{s}")
                    dve.wait_ge(sem_act, plan[f"amp_{s}"])
                    dve.wait_ge(sem_dve, plan[f"s12_{s}"])  # self RAW
                    nc.vector.tensor_mul(
                        apr[s][:], s12[s][:, 0, :], amp[s][:]
                    ).then_inc(sem_dve, 1)
                    bump(f"apr_{s}")
                    nc.vector.tensor_mul(
                        ppr[s][:], s12[s][:, 1, :], fi[s][:]
                    ).then_inc(sem_dve, 1)
                    bump(f"ppr_{s}")
                    dve.wait_ge(sem_act, plan[f"sinp_{s}"])
                    dve.wait_ge(sem_dve, plan[f"apr_{s}"])  # self RAW
                    nc.vector.tensor_mul(
                        zr[s][:], apr[s][:], cosp[s][:]
                    ).then_inc(sem_dve, 1)
                    bump()
                    nc.vector.tensor_mul(
                        zi[s][:], apr[s][:], sinp[s][:]
                    ).then_inc(sem_dve, 1)
                    bump(f"zi_{s}")

                def add_unit(s, u, first_of_sample):
                    b = buf_of[(s, u)]
                    cg = u // nhalf
                    if first_of_sample:
                        dve.wait_ge(sem_act, plan[f"xi_ev_{s}"])
                    nc.vector.tensor_scalar_add(
                        xb[b][:], xb[b][:], xi[s][:, cg:cg + 1]
                    ).then_inc(sem_cons, 1)
                    cons_ct["n"] += 1
                    add_ord[(s, u)] = cons_ct["n"]

                # ---- emission ----
                for u in range(tps):
                    reduce_unit(0, u)
                gcol_reduce(0)
                # s1 spare-buffer reduces while PE/ACT run sample-0 stats
                for u in range(nspare):
                    reduce_unit(1, u)
                stats_dve(0)
                for u in range(tps):
                    add_unit(0, u, u == 0)
                for u in range(nspare, tps):
                    reduce_unit(1, u)
                gcol_reduce(1)
                stats_dve(1)
                for u in range(tps):
                    add_unit(1, u, u == 0)

            @block.scalar
            def _(act):
                nv = act_v

                def bump(tag=None):
                    nv["n"] += 1
                    if tag:
                        assert plan[tag] == nv["n"], (tag, plan[tag], nv["n"])

                # const loads on the otherwise-idle ACT HWDGE ring so x
                # streaming starts immediately on the SP ring
                for dram, sbuf in (
                    (cos_d, cos_sb), (sin_d, sin_sb), (w1_d, w1_sb),
                    (w2_d, w2_sb), (b_d, b_sb),
                ):
                    nc.scalar.dma_start(out=sbuf[:], in_=dram[:]).then_inc(
                        sem_cst, 16
                    )
                act.wait_ge(sem_dve, plan["memset"])
                for s in range(NS):
                    act.wait_ge(sem_pe, plan[f"fwd_{s}"])
                    nc.scalar.mul(fr[s][:], fwd_ps[s][:, 0, :], 1.0 / hw)
                    bump()
                    nc.scalar.mul(fi[s][:], fwd_ps[s][:, 1, :], 1.0 / hw).then_inc(
                        sem_act, 2
                    )
                    bump(f"fi_ev_{s}")
                    act.wait_ge(sem_dve, plan[f"z12_{s}"])
                    nc.scalar.activation(
                        r2[s][:], z12[s][:], _AF.Relu, scale=-1.0
                    ).then_inc(sem_act, 1)
                    bump(f"r2_{s}")
                    act.wait_ge(sem_dve, plan[f"u0_{s}"])
                    nc.scalar.activation(amp[s][:], u0[s][:], _AF.Sqrt).then_inc(
                        sem_act, 1
                    )
                    bump(f"amp_{s}")
                    act.wait_ge(sem_dve, plan[f"ppr_{s}"])
                    nc.scalar.activation(
                        cosp[s][:], ppr[s][:], _AF.Sin, bias=halfpi[:]
                    )
                    bump()
                    nc.scalar.activation(sinp[s][:], ppr[s][:], _AF.Sin).then_inc(
                        sem_act, 2
                    )
                    bump(f"sinp_{s}")
                    act.wait_ge(sem_pe, plan[f"inv_{s}"])  # inverse mm done
                    nc.scalar.mul(xi[s][:], xi_ps[s][:], 1.0 / C).then_inc(
                        sem_act, 1
                    )
                    bump(f"xi_ev_{s}")

            @block.tensor
            def _(pe):
                pe.wait_ge(sem_cst, 16 * n_const)  # consts resident
                for s in range(NS):
                    # fwd s then inv s so xi_s lands as early as possible
                    pe.wait_ge(sem_dve, plan[f"gcol{s}"])
                    last = None
                    for t, mat in enumerate((cos_sb, sin_sb, w1_sb, w2_sb)):
                        for kg in range(G):
                            for cg in range(G):
                                last = nc.tensor.matmul(
                                    fwd_ps[s][:, t, kg:kg + 1],
                                    mat[:, cg, kg * P:(kg + 1) * P],
                                    gcol[s][:, cg:cg + 1],
                                    start=(cg == 0),
                                    stop=(cg == G - 1),
                                )
                    last.then_inc(sem_pe, 1)  # fwd_s = 2s+1
                    pe.wait_ge(sem_dve, plan[f"zi_{s}"])
                    last = None
                    for cg in range(G):
                        for kg in range(G):
                            nc.tensor.matmul(
                                xi_ps[s][:, cg:cg + 1],
                                cos_sb[:, kg, cg * P:(cg + 1) * P],
                                zr[s][:, kg:kg + 1],
                                start=(kg == 0),
                                stop=False,
                            )
                            last = nc.tensor.matmul(
                                xi_ps[s][:, cg:cg + 1],
                                sin_sb[:, kg, cg * P:(cg + 1) * P],
                                zi[s][:, kg:kg + 1],
                                start=False,
                                stop=(kg == G - 1),
                            )
                    last.then_inc(sem_pe, 1)  # inv_s = 2s+2

            @block.sync
            def _(sp):
                for u in range(tps):  # sample 0
                    sp.dma_start(
                        out=xb[u][:], in_=unit_ap(x_in, 0, u)
                    ).then_inc(ld[u], 16)
                for u in range(nspare):  # sample 1 head -> spare bufs
                    sp.dma_start(
                        out=xb[tps + u][:], in_=unit_ap(x_in, 1, u)
                    ).then_inc(ld[tps + u], 16)
                for u in range(nspare, tps):  # sample 1 tail -> reused bufs
                    b = u - nspare
                    sp.wait_ge(st[b], 16)  # s0's store from buf b done
                    sp.dma_start(
                        out=xb[b][:], in_=unit_ap(x_in, 1, u)
                    ).then_inc(ld[b], 16)

            @block.gpsimd
            def _(gp):
                for s in range(NS):
                    for u in range(tps):
                        b = buf_of[(s, u)]
                        gp.wait_ge(sem_cons, add_ord[(s, u)])
                        d = gp.dma_start(
                            out=unit_ap(x_out, s, u), in_=xb[b][:]
                        )
                        if s == 0 and b < tps - nspare:
                            d.then_inc(st[b], 16)  # unblocks s1's reload
                        else:
                            d.then_inc(st_misc, 16)  # unwaited

    return nc


_NC_CACHE = None


def _get_program():
    global _NC_CACHE
    if _NC_CACHE is None:
        _NC_CACHE = _build_program()
    return _NC_CACHE


def _host_constants():
    idx = np.arange(C)
    th = (2.0 * np.pi / C) * np.outer(idx, idx)
    cosm = np.cos(th).astype(np.float32)
    sinn = (-np.sin(th)).astype(np.float32)
    # [p, g, k] layout with row c = g*128+p
    to_pgk = lambda m: np.ascontiguousarray(
        m.reshape(G, P, C).transpose(1, 0, 2)
    ).astype(_NP_BF16)
    return to_pgk(cosm), to_pgk(sinn)


_CONSTS_CACHE = None


def make_in_maps(inputs, hw=HW):
    """Shard + preprocess inputs into 8 per-core input maps."""
    global _CONSTS_CACHE
    if _CONSTS_CACHE is None:
        _CONSTS_CACHE = _host_constants()
    cos_pgk, sin_pgk = _CONSTS_CACHE

    x = np.asarray(inputs["x"])
    W1 = np.asarray(inputs["W1"], dtype=np.float32)
    W2 = np.asarray(inputs["W2"], dtype=np.float32)
    b1 = np.asarray(inputs["b1"], dtype=np.float32)
    b2 = np.asarray(inputs["b2"], dtype=np.float32)

    # fold the 1/HW mean normalization into the linear-layer weights
    w1t = np.ascontiguousarray(
        (W1.T / hw).reshape(G, P, C).transpose(1, 0, 2)
    ).astype(_NP_BF16)
    w2t = np.ascontiguousarray(
        (W2.T / hw).reshape(G, P, C).transpose(1, 0, 2)
    ).astype(_NP_BF16)
    bvec = np.ascontiguousarray(
        np.stack([b1.reshape(G, P), b2.reshape(G, P)]).transpose(2, 0, 1),
        dtype=np.float32,
    )  # [P, 2, G]

    xs = np.ascontiguousarray(x, dtype=np.float16).reshape(NCORES, NS, C, hw)
    return [
        {
            "x": xs[i],
            "cosm": cos_pgk,
            "sinn": sin_pgk,
            "w1t": w1t,
            "w2t": w2t,
            "bvec": bvec,
        }
        for i in range(NCORES)
    ]


def _run(inputs, trace=False, trace_kwargs=None):
    in_maps = make_in_maps(inputs)
    nc = _get_program()
    res = run_bass_kernel_spmd(
        nc,
        in_maps,
        list(range(NCORES)),
        trace=trace,
        **(trace_kwargs or {}),
    )
    out = np.stack([r["out"] for r in res.results])
    return out.reshape(N, C, H, W).astype(np.float32), res


def kernel(**inputs) -> np.ndarray:
    out, _ = _run(inputs, trace=False)
    return out


# revision 12
# speedup vs baseline: 2.4672x; 1.0749x over previous
"""Trainium2 Bass kernel for nn_CFTL_60327110640070.

out = x + ifft_c( fused(fft_c(mean_hw(x)), g@W1.T+b1, g@W2.T+b2) )  broadcast over HW

Strategy (pure data parallel, 8 cores, 2 samples each, fp16 streaming):
  x is uploaded to the device as fp16 (halves load traffic; rel-err ~2e-4
  is far inside the 2e-2 gate) and the output is written as fp16 and
  upcast to fp32 on the host (halves store traffic). A full sample
  (16 x [128,4096] fp16 tiles = 128 KiB/partition) stays resident in
  SBUF, so x is read exactly once -- no second pass.

  per sample: load 16 tiles; the HW-mean runs as a per-channel-group
  elementwise tree on DVE (3 tensor_tensor adds into a ping-pong acc
  tile; packed-fp16 2x mode, ~1.2us/tile vs 3.7us for InstTensorReduce
  which has no 2x mode) followed by an ACT Copy-with-accum_out that
  emits the [P,1] group sum directly (ACT is otherwise idle). Then PE
  DFT/linear matmuls in bf16 (fp32 PE runs as 2 half-speed passes; bf16
  is 4x => LDWEIGHTS 28ns vs 214ns), the small DVE/ACT stats chain,
  xi, DVE in-place per-channel add, GP store. Sample 1's first 5 tiles
  load into spare buffers during sample 0's stats/adds; its remaining
  11 reuse sample-0 buffers as stores drain.

Raw bass (no Tile): all waits are standalone wait_ge on the issuing
engine; DMAs carry only their completion-sem update. Each DMA sem has at
most one in-flight DMA at a time (enforced by the data deps), so
cumulative 16*k waits are race-free against the 16 per-engine
micro-increments.

All DFT/weight matrices are pre-transposed/pre-scaled on host so no
on-device transposes are needed (cos/-sin DFT matrices are symmetric).
"""

import sys
from contextlib import ExitStack

for _p in ("/opt/trn_rl_repo", "/root/.axon_site/_ro/trn_rl_repo"):
    if _p not in sys.path:
        sys.path.append(_p)

import numpy as np

import concourse.bass as bass
from concourse import mybir
from concourse.bass_utils import run_bass_kernel_spmd

# Problem geometry (hardcoded per contract)
N, C, H, W = 16, 512, 128, 128
HW = H * W
NCORES = 8
NS = N // NCORES          # samples per core = 2
P = 128                   # SBUF partitions
G = C // P                # channel groups = 4
FREE = 4096               # free-dim tile size for streaming x
NSPARE = 5                # extra unit buffers for cross-sample overlap

_FP32 = mybir.dt.float32
_FP16 = mybir.dt.float16
_BF16 = mybir.dt.bfloat16
_AF = mybir.ActivationFunctionType
_NP_BF16 = np.dtype(mybir.dt.np(_BF16))


def _build_program(free=FREE, hw=HW, nspare=NSPARE) -> bass.Bass:
    nhalf = hw // free           # tiles per (sample, group) = 4
    tps = G * nhalf              # x tiles (units) per sample = 16
    n_const = 5
    nbuf = tps + nspare          # 21 unit buffers

    # buffer assignment: s0 units -> bufs 0..tps-1; s1 units 0..nspare-1 ->
    # spare bufs; s1 units nspare.. -> bufs 0..tps-nspare-1 (after s0 store)
    buf_of = {}
    for u in range(tps):
        buf_of[(0, u)] = u
    for u in range(nspare):
        buf_of[(1, u)] = tps + u
    for u in range(nspare, tps):
        buf_of[(1, u)] = u - nspare

    nc = bass.Bass(dynamic_dma_scratch_size=8192)

    x_in = nc.dram_tensor("x", [NS, C, hw], _FP16, kind="ExternalInput")
    x_out = nc.dram_tensor("out", [NS, C, hw], _FP16, kind="ExternalOutput")
    # host pre-layouts: [p, g, k] with row index c = g*128+p
    cos_d = nc.dram_tensor("cosm", [P, G, C], _BF16, kind="ExternalInput")
    sin_d = nc.dram_tensor("sinn", [P, G, C], _BF16, kind="ExternalInput")
    w1_d = nc.dram_tensor("w1t", [P, G, C], _BF16, kind="ExternalInput")
    w2_d = nc.dram_tensor("w2t", [P, G, C], _BF16, kind="ExternalInput")
    b_d = nc.dram_tensor("bvec", [P, 2, G], _FP32, kind="ExternalInput")

    def unit_ap(dram, s, u):
        cg, h = divmod(u, nhalf)
        return dram[s, cg * P:(cg + 1) * P, h * free:(h + 1) * free]

    with ExitStack() as ctx:
        sb = lambda shape, name, dt=_FP32: ctx.enter_context(
            nc.sbuf_tensor(name, shape, dt)
        )
        ps = lambda shape, name: ctx.enter_context(
            nc.psum_tensor(name, shape, _FP32)
        )
        sem = lambda name: ctx.enter_context(nc.semaphore(name))

        cos_sb = sb([P, G, C], "cos_sb", _BF16)
        sin_sb = sb([P, G, C], "sin_sb", _BF16)
        w1_sb = sb([P, G, C], "w1_sb", _BF16)
        w2_sb = sb([P, G, C], "w2_sb", _BF16)
        b_sb = sb([P, 2, G], "b_sb")
        halfpi = sb([P, 1], "halfpi")

        xb = [sb([P, free], f"xb{i}", _FP16) for i in range(nbuf)]
        acc = [sb([P, free], f"acc{i}", _FP16) for i in range(2)]  # ping-pong

        gcolf = [sb([P, G], f"gcolf{s}") for s in range(NS)]       # fp32 sums
        gcol = [sb([P, G], f"gcol{s}", _BF16) for s in range(NS)]  # for PE
        fr = [sb([P, G], f"fr{s}") for s in range(NS)]
        fi = [sb([P, G], f"fi{s}") for s in range(NS)]
        z12 = [sb([P, 2, G], f"z12_{s}") for s in range(NS)]
        r2 = [sb([P, 2, G], f"r2_{s}") for s in range(NS)]
        s12 = [sb([P, 2, G], f"s12_{s}") for s in range(NS)]
        u0 = [sb([P, G], f"u0_{s}") for s in range(NS)]
        amp = [sb([P, G], f"amp{s}") for s in range(NS)]
        apr = [sb([P, G], f"apr{s}") for s in range(NS)]
        cosp = [sb([P, G], f"cosp{s}") for s in range(NS)]
        sinp = [sb([P, G], f"sinp{s}") for s in range(NS)]
        xi = [sb([P, G], f"xi{s}") for s in range(NS)]
        zr = [sb([P, G], f"zr{s}", _BF16) for s in range(NS)]
        zi = [sb([P, G], f"zi{s}", _BF16) for s in range(NS)]
        # aliases: each write is sem-ordered after the previous tenant's
        # last read (same per-sample op order as validated baseline)
        u1 = amp    # u1 read by u0-add; amp written after (waits u0 done)
        ppr = fr    # fr dead after u0-mul; ppr written next

        fwd_ps = [ps([P, 4, G], f"fwd_ps{s}") for s in range(NS)]
        xi_ps = [ps([P, G], f"xi_ps{s}") for s in range(NS)]

        ld = [sem(f"ld{b}") for b in range(nbuf)]
        st = [sem(f"st{b}") for b in range(tps - nspare)]  # waited reloads
        st_misc = sem("st_misc")    # unwaited store completions
        sem_cst = sem("sem_cst")    # const load completions (+16 each)
        sem_cons = sem("sem_cons")  # DVE xi-add done (+1, ordinal s*tps+u+1)
        sem_tree = sem("sem_tree")  # DVE tree-add count (+1 each)
        sem_dve = sem("sem_dve")    # DVE stats milestones
        sem_act = sem("sem_act")    # ACT milestones (copies + stats)
        sem_pe = sem("sem_pe")      # PE matmul groups

        # planned sem values after named ops (any stream may reference any)
        # DVE sem_dve: memset=1; stats ops per sample: 9 (z12,s12,u0m,u1m,
        #   u0,apr,ppr,zr,zi) -> base = 1 + 9*s
        # DVE sem_tree: 12 tree adds per sample; tree (s,cg) done at
        #   12*s + 3*(cg+1)
        # ACT sem_act per sample (12): gc0..gc3 (+1 each), gcol16 (+1),
        #   fr,fi (+2 at fi), r2 (+1), amp (+1), cosp,sinp (+2 at sinp),
        #   xi (+1)
        plan = {"memset": 1}
        for s in range(NS):
            base = 1 + 9 * s
            plan[f"z12_{s}"] = base + 1
            plan[f"s12_{s}"] = base + 2
            plan[f"u1m_{s}"] = base + 4
            plan[f"u0_{s}"] = base + 5
            plan[f"apr_{s}"] = base + 6
            plan[f"ppr_{s}"] = base + 7
            plan[f"zi_{s}"] = base + 9
            for cg in range(G):
                plan[f"tree_{s}_{cg}"] = 12 * s + 3 * (cg + 1)
                plan[f"gc_{s}_{cg}"] = 12 * s + cg + 1
            plan[f"gcol16_{s}"] = 12 * s + 5
            plan[f"fi_ev_{s}"] = 12 * s + 7
            plan[f"r2_{s}"] = 12 * s + 8
            plan[f"amp_{s}"] = 12 * s + 9
            plan[f"sinp_{s}"] = 12 * s + 11
            plan[f"xi_ev_{s}"] = 12 * s + 12
            plan[f"fwd_{s}"] = 2 * s + 1
            plan[f"inv_{s}"] = 2 * s + 2

        dve_v = {"n": 0}
        act_v = {"n": 0}
        tree_v = {"n": 0}

        with nc.Block() as block:

            @block.vector
            def _(dve):
                def bump(counter, tag=None):
                    counter["n"] += 1
                    if tag:
                        assert plan[tag] == counter["n"], (
                            tag, plan[tag], counter["n"],
                        )

                nc.vector.memset(halfpi[:], float(np.pi / 2)).then_inc(sem_dve, 1)
                bump(dve_v, "memset")

                def ld_wait(s, u):
                    b = buf_of[(s, u)]
                    gen2 = s == 1 and u >= nspare
                    dve.wait_ge(ld[b], 32 if gen2 else 16)
                    return xb[b]

                def tree_cg(s, cg):
                    """3 elementwise adds of the group's 4 tiles into a
                    ping-pong acc; ACT then turns acc into the group sum."""
                    t = s * G + cg  # global tree index; acc[t % 2]
                    a = acc[t % 2]
                    if t >= 2:
                        # previous tenant of this acc: tree t-2's ACT copy
                        ps_, pcg = divmod(t - 2, G)
                        dve.wait_ge(sem_act, plan[f"gc_{ps_}_{pcg}"])
                    b0 = ld_wait(s, cg * nhalf + 0)
                    b1 = ld_wait(s, cg * nhalf + 1)
                    nc.vector.tensor_add(a[:], b0[:], b1[:]).then_inc(
                        sem_tree, 1
                    )
                    bump(tree_v)
                    b2 = ld_wait(s, cg * nhalf + 2)
                    nc.vector.tensor_add(a[:], a[:], b2[:]).then_inc(
                        sem_tree, 1
                    )
                    bump(tree_v)
                    b3 = ld_wait(s, cg * nhalf + 3)
                    nc.vector.tensor_add(a[:], a[:], b3[:]).then_inc(
                        sem_tree, 1
                    )
                    bump(tree_v, f"tree_{s}_{cg}")

                def stats_dve(s):
                    dve.wait_ge(sem_pe, plan[f"fwd_{s}"])
                    if s == 0:
                        dve.wait_ge(sem_cst, 16 * n_const)  # b_sb resident
                    nc.vector.tensor_add(
                        z12[s][:], fwd_ps[s][:, 2:4, :], b_sb[:]
                    ).then_inc(sem_dve, 1)
                    bump(dve_v, f"z12_{s}")
                    # leaky_relu(z) = z + 0.99*relu(-z)
                    dve.wait_ge(sem_act, plan[f"r2_{s}"])
                    dve.wait_ge(sem_dve, plan[f"z12_{s}"])  # self RAW
                    nc.vector.scalar_tensor_tensor(
                        out=s12[s][:], in0=r2[s][:], scalar=0.99, in1=z12[s][:],
                        op0=mybir.AluOpType.mult, op1=mybir.AluOpType.add,
                    ).then_inc(sem_dve, 1)
                    bump(dve_v, f"s12_{s}")
                    dve.wait_ge(sem_act, plan[f"fi_ev_{s}"])
                    nc.vector.tensor_mul(u0[s][:], fr[s][:], fr[s][:]).then_inc(
                        sem_dve, 1
                    )
                    bump(dve_v)
                    nc.vector.tensor_mul(u1[s][:], fi[s][:], fi[s][:]).then_inc(
                        sem_dve, 1
                    )
                    bump(dve_v, f"u1m_{s}")
                    dve.wait_ge(sem_dve, plan[f"u1m_{s}"])  # self RAW u0/u1
                    nc.vector.tensor_add(u0[s][:], u0[s][:], u1[s][:]).then_inc(
                        sem_dve, 1
                    )
                    bump(dve_v, f"u0_{s}")
                    dve.wait_ge(sem_act, plan[f"amp_{s}"])
                    dve.wait_ge(sem_dve, plan[f"s12_{s}"])  # self RAW
                    nc.vector.tensor_mul(
                        apr[s][:], s12[s][:, 0, :], amp[s][:]
                    ).then_inc(sem_dve, 1)
                    bump(dve_v, f"apr_{s}")
                    nc.vector.tensor_mul(
                        ppr[s][:], s12[s][:, 1, :], fi[s][:]
                    ).then_inc(sem_dve, 1)
                    bump(dve_v, f"ppr_{s}")
                    dve.wait_ge(sem_act, plan[f"sinp_{s}"])
                    dve.wait_ge(sem_dve, plan[f"apr_{s}"])  # self RAW
                    nc.vector.tensor_mul(
                        zr[s][:], apr[s][:], cosp[s][:]
                    ).then_inc(sem_dve, 1)
                    bump(dve_v)
                    nc.vector.tensor_mul(
                        zi[s][:], apr[s][:], sinp[s][:]
                    ).then_inc(sem_dve, 1)
                    bump(dve_v, f"zi_{s}")

                add_ord = {}

                def add_unit(s, u, first_of_sample):
                    b = buf_of[(s, u)]
                    cg = u // nhalf
                    if first_of_sample:
                        dve.wait_ge(sem_act, plan[f"xi_ev_{s}"])
                    nc.vector.tensor_scalar_add(
                        xb[b][:], xb[b][:], xi[s][:, cg:cg + 1]
                    ).then_inc(sem_cons, 1)
                    add_ord[(s, u)] = len(add_ord) + 1

                # ---- emission ----
                for cg in range(G):
                    tree_cg(0, cg)
                # s1's first group lands in spare bufs early; reduce it while
                # ACT/PE run sample-0's transform
                tree_cg(1, 0)
                stats_dve(0)
                for u in range(tps):
                    add_unit(0, u, u == 0)
                for cg in range(1, G):
                    tree_cg(1, cg)
                stats_dve(1)
                for u in range(tps):
                    add_unit(1, u, u == 0)

            @block.scalar
            def _(act):
                def bump(tag=None):
                    act_v["n"] += 1
                    if tag:
                        assert plan[tag] == act_v["n"], (
                            tag, plan[tag], act_v["n"],
                        )

                # const loads on the otherwise-idle ACT HWDGE ring so x
                # streaming starts immediately on the SP ring
                for dram, sbuf in (
                    (cos_d, cos_sb), (sin_d, sin_sb), (w1_d, w1_sb),
                    (w2_d, w2_sb), (b_d, b_sb),
                ):
                    nc.scalar.dma_start(out=sbuf[:], in_=dram[:]).then_inc(
                        sem_cst, 16
                    )
                act.wait_ge(sem_dve, plan["memset"])
                for s in range(NS):
                    # group sums: Copy-with-accum on the acc tiles
                    for cg in range(G):
                        act.wait_ge(sem_tree, plan[f"tree_{s}_{cg}"])
                        t = s * G + cg
                        nc.scalar.activation(
                            acc[t % 2][:], acc[t % 2][:], _AF.Copy,
                            accum_out=gcolf[s][:, cg:cg + 1],
                        ).then_inc(sem_act, 1)
                        bump(f"gc_{s}_{cg}")
                    # bf16 copy for the PE matmuls
                    with nc.allow_low_precision(reason="bf16 gcol for bf16 PE"):
                        nc.scalar.copy(gcol[s][:], gcolf[s][:]).then_inc(
                            sem_act, 1
                        )
                    bump(f"gcol16_{s}")
                    act.wait_ge(sem_pe, plan[f"fwd_{s}"])
                    nc.scalar.mul(fr[s][:], fwd_ps[s][:, 0, :], 1.0 / hw)
                    bump()
                    nc.scalar.mul(fi[s][:], fwd_ps[s][:, 1, :], 1.0 / hw).then_inc(
                        sem_act, 2
                    )
                    bump(f"fi_ev_{s}")
                    act.wait_ge(sem_dve, plan[f"z12_{s}"])
                    nc.scalar.activation(
                        r2[s][:], z12[s][:], _AF.Relu, scale=-1.0
                    ).then_inc(sem_act, 1)
                    bump(f"r2_{s}")
                    act.wait_ge(sem_dve, plan[f"u0_{s}"])
                    nc.scalar.activation(amp[s][:], u0[s][:], _AF.Sqrt).then_inc(
                        sem_act, 1
                    )
                    bump(f"amp_{s}")
                    act.wait_ge(sem_dve, plan[f"ppr_{s}"])
                    nc.scalar.activation(
                        cosp[s][:], ppr[s][:], _AF.Sin, bias=halfpi[:]
                    )
                    bump()
                    nc.scalar.activation(sinp[s][:], ppr[s][:], _AF.Sin).then_inc(
                        sem_act, 2
                    )
                    bump(f"sinp_{s}")
                    act.wait_ge(sem_pe, plan[f"inv_{s}"])  # inverse mm done
                    nc.scalar.mul(xi[s][:], xi_ps[s][:], 1.0 / C).then_inc(
                        sem_act, 1
                    )
                    bump(f"xi_ev_{s}")

            @block.tensor
            def _(pe):
                pe.wait_ge(sem_cst, 16 * n_const)  # consts resident
                for s in range(NS):
                    # fwd s then inv s so xi_s lands as early as possible
                    pe.wait_ge(sem_act, plan[f"gcol16_{s}"])
                    last = None
                    for t, mat in enumerate((cos_sb, sin_sb, w1_sb, w2_sb)):
                        for kg in range(G):
                            for cg in range(G):
                                last = nc.tensor.matmul(
                                    fwd_ps[s][:, t, kg:kg + 1],
                                    mat[:, cg, kg * P:(kg + 1) * P],
                                    gcol[s][:, cg:cg + 1],
                                    start=(cg == 0),
                                    stop=(cg == G - 1),
                                )
                    last.then_inc(sem_pe, 1)  # fwd_s = 2s+1
                    pe.wait_ge(sem_dve, plan[f"zi_{s}"])
                    last = None
                    for cg in range(G):
                        for kg in range(G):
                            nc.tensor.matmul(
                                xi_ps[s][:, cg:cg + 1],
                                cos_sb[:, kg, cg * P:(cg + 1) * P],
                                zr[s][:, kg:kg + 1],
                                start=(kg == 0),
                                stop=False,
                            )
                            last = nc.tensor.matmul(
                                xi_ps[s][:, cg:cg + 1],
                                sin_sb[:, kg, cg * P:(cg + 1) * P],
                                zi[s][:, kg:kg + 1],
                                start=False,
                                stop=(kg == G - 1),
                            )
                    last.then_inc(sem_pe, 1)  # inv_s = 2s+2

            @block.sync
            def _(sp):
                for u in range(tps):  # sample 0
                    sp.dma_start(
                        out=xb[u][:], in_=unit_ap(x_in, 0, u)
                    ).then_inc(ld[u], 16)
                for u in range(nspare):  # sample 1 head -> spare bufs
                    sp.dma_start(
                        out=xb[tps + u][:], in_=unit_ap(x_in, 1, u)
                    ).then_inc(ld[tps + u], 16)
                for u in range(nspare, tps):  # sample 1 tail -> reused bufs
                    b = u - nspare
                    sp.wait_ge(st[b], 16)  # s0's store from buf b done
                    sp.dma_start(
                        out=xb[b][:], in_=unit_ap(x_in, 1, u)
                    ).then_inc(ld[b], 16)

            @block.gpsimd
            def _(gp):
                for s in range(NS):
                    for u in range(tps):
                        b = buf_of[(s, u)]
                        gp.wait_ge(sem_cons, s * tps + u + 1)  # add done
                        d = gp.dma_start(
                            out=unit_ap(x_out, s, u), in_=xb[b][:]
                        )
                        if s == 0 and b < tps - nspare:
                            d.then_inc(st[b], 16)  # unblocks s1's reload
                        else:
                            d.then_inc(st_misc, 16)  # unwaited

    return nc


_NC_CACHE = None


def _get_program():
    global _NC_CACHE
    if _NC_CACHE is None:
        _NC_CACHE = _build_program()
    return _NC_CACHE


def _host_constants():
    idx = np.arange(C)
    th = (2.0 * np.pi / C) * np.outer(idx, idx)
    cosm = np.cos(th).astype(np.float32)
    sinn = (-np.sin(th)).astype(np.float32)
    # [p, g, k] layout with row index c = g*128+p
    to_pgk = lambda m: np.ascontiguousarray(
        m.reshape(G, P, C).transpose(1, 0, 2)
    ).astype(_NP_BF16)
    return to_pgk(cosm), to_pgk(sinn)


_CONSTS_CACHE = None


def make_in_maps(inputs, hw=HW):
    """Shard + preprocess inputs into 8 per-core input maps."""
    global _CONSTS_CACHE
    if _CONSTS_CACHE is None:
        _CONSTS_CACHE = _host_constants()
    cos_pgk, sin_pgk = _CONSTS_CACHE

    x = np.asarray(inputs["x"])
    W1 = np.asarray(inputs["W1"], dtype=np.float32)
    W2 = np.asarray(inputs["W2"], dtype=np.float32)
    b1 = np.asarray(inputs["b1"], dtype=np.float32)
    b2 = np.asarray(inputs["b2"], dtype=np.float32)

    # fold the 1/HW mean normalization into the linear-layer weights
    w1t = np.ascontiguousarray(
        (W1.T / hw).reshape(G, P, C).transpose(1, 0, 2)
    ).astype(_NP_BF16)
    w2t = np.ascontiguousarray(
        (W2.T / hw).reshape(G, P, C).transpose(1, 0, 2)
    ).astype(_NP_BF16)
    bvec = np.ascontiguousarray(
        np.stack([b1.reshape(G, P), b2.reshape(G, P)]).transpose(2, 0, 1),
        dtype=np.float32,
    )  # [P, 2, G]

    xs = np.ascontiguousarray(x, dtype=np.float16).reshape(NCORES, NS, C, hw)
    return [
        {
            "x": xs[i],
            "cosm": cos_pgk,
            "sinn": sin_pgk,
            "w1t": w1t,
            "w2t": w2t,
            "bvec": bvec,
        }
        for i in range(NCORES)
    ]


def _run(inputs, trace=False, trace_kwargs=None):
    in_maps = make_in_maps(inputs)
    nc = _get_program()
    res = run_bass_kernel_spmd(
        nc,
        in_maps,
        list(range(NCORES)),
        trace=trace,
        **(trace_kwargs or {}),
    )
    out = np.stack([r["out"] for r in res.results])
    return out.reshape(N, C, H, W).astype(np.float32), res


def kernel(**inputs) -> np.ndarray:
    out, _ = _run(inputs, trace=False)
    return out
